# revision 51
# baseline (speedup 1.0000x reference)
"""GAT (2-layer graph attention network) on 8 Trainium2 NeuronCores.

Strategy (1D node partition): each core owns R = N/8 rows (nodes).

Layer 1:
  - Wh plus the per-node score projections s- = Wh @ a_minus are computed
    from LOCAL rows only (one fused matmul chain per row tile; the
    projection weights W@a_half are precomputed on host), packed into an
    AllGather payload laid out [Wh_h0 |1| Wh_h1 |1| Wh_h2 |1| Wh_h3 |1|
    s-_0..3 | pad] so that each head's aggregation can stream a contiguous
    [Wh_h | ones] block.
  - Scores e[j, i] = leaky_relu(s+_i + s-_j + maskbias) are built by a
    fused custom DVE op per tile (mask bias as fp8_e4m3, exact for {0,-96});
    exp on the scalar engine in half-group batches.
  - Aggregation uses z as the matmul STATIONARY operand streaming the
    gathered [Wh_h | 1] columns: PSUM picks up both the softmax numerator
    (128 cols) and the denominator (col 128) in one pass — no separate
    denominator matmuls.  Each accumulation chain owns a full 2KB PSUM
    bank (interleaved chains sharing a bank corrupt each other on HW).
    Normalize (Act Copy with per-partition reciprocal scale) + elu + PE
    transpose produce feature-major h for layer 2.
Between layers: AllGather of [Wh2(64) | 1 | s2-] payload (68 cols/tile).
Layer 2: same fused-score pipeline; denominator rides as column 64 of the
stationary operand (m=65 <= 128).

DMA discipline: bulk loads ride the Pool (SWDGE) queue whose dispatch is
cheap and whose desc-gen runs on the idle Pool engine; gather readbacks are
chunked so attention starts as soon as the first chunk lands; the late mask
chunks dispatch from the Act queue after s1b so they cannot crowd the
critical readback on the DMA engines.

Numerics: matmuls in bf16 (fp32 PSUM accumulate); mask handled as additive
-96 before leaky_relu: masked contribution < 1e-8 relative.
"""

import math
from contextlib import ExitStack
from dataclasses import dataclass

import numpy as np
import ml_dtypes

import concourse.bass as bass
import concourse.mybir as mybir
import concourse.tile as tile
from concourse import bacc
from concourse.bass_utils import run_bass_kernel_spmd

BF16 = ml_dtypes.bfloat16
ALPHA = 0.2
MASKBIAS = -96.0

# --------------------------------------------------------------------------
# Custom fused DVE ops (registered into concourse.dve_ops at import time)
# --------------------------------------------------------------------------

import concourse.dve_ops as dve_ops
from concourse.dve_spec import (
    Spec, Src0, Src1, C0, Zero, lower, maxx, select, _has_src1,
)
from concourse.dve_uop import DveOpSpec


def _make_specs():
    # out = max(y, alpha*y), y = (in0 + s0) + in1
    #   in0 = s1 broadcast [P, R]; s0 = s2 per-partition [P, 1];
    #   in1 = additive mask bias {0, -100}; imm2 = alpha
    from concourse.dve_spec import C2
    _y = (Src0 + C0) + Src1

    def _score_ref(in0, in1, s0, s1, imm2):
        y = in0.astype(np.float32) + s0 + in1.astype(np.float32)
        return np.maximum(y, y * imm2)

    score = Spec(body=maxx(_y, _y * C2), reference=_score_ref)

    # out = in0 > 0 ? in0 : in1 - s0   (elu with in1 = exp(in0), s0 = 1.0)
    def _elu_ref(in0, in1, s0, s1, imm2):
        x = in0.astype(np.float32)
        return np.where(x > 0, x, in1.astype(np.float32) - s0)

    elu = Spec(body=select(Src0 > Zero, Src0, Src1 - C0), reference=_elu_ref)
    return score, elu


def _register(name, spec):
    if name in dve_ops._SUB_OPCODE_FOR_NAME:
        for op in dve_ops.OPS:
            if op.name == name:
                return op
    row = max(dve_ops._SUB_OPCODE_FOR_NAME.values()) + 1
    assert row < 0x20
    shas = {}
    for ver in ("v3", "v4"):
        uops = lower(spec, ver=ver)
        shas[ver] = DveOpSpec(
            name=name, opcode=row, uops=uops, rd1_en=_has_src1(spec)
        ).sha(ver)
    op = dve_ops.DveOp(name, spec, subdim=False, uops_sha=shas)
    dve_ops.OPS.append(op)
    dve_ops.CUSTOM_DVE_SPECS[name] = spec
    dve_ops._SUB_OPCODE_FOR_NAME[name] = row
    return op


_SCORE_SPEC, _ELU_SPEC = _make_specs()
SCORE_LRELU = _register("SCORE_LRELU_GAT", _SCORE_SPEC)
ELU_SEL = _register("ELU_SEL_GAT", _ELU_SPEC)


# --------------------------------------------------------------------------
# Kernel configuration
# --------------------------------------------------------------------------

@dataclass(frozen=True)
class Cfg:
    N: int = 4096      # nodes
    C: int = 512       # input feature dim
    H: int = 128       # hidden per head (must be 128)
    HEADS: int = 4
    F2: int = 64       # output dim
    CORES: int = 8
    GRP: int = 8       # j-tiles per batched exp

    @property
    def R(self): return self.N // self.CORES          # rows per core
    @property
    def JT(self): return self.N // 128                # j tiles
    @property
    def CT(self): return self.C // 128                # input-feature tiles
    @property
    def HH(self): return self.HEADS * self.H          # layer-1 out features
    @property
    def CT2(self): return self.HH // 128              # layer-2 contraction tiles
    @property
    def RT(self): return self.R // 128                # local row tiles
    @property
    def PW1(self): return self.HEADS * 129 + self.HEADS  # 520 payload cols
    @property
    def PAY(self): return self.F2 + 4                 # L2 payload (64|1|s2|pad)
    @property
    def NG(self): return self.JT // self.GRP


FULL = Cfg()


# --------------------------------------------------------------------------
# Device program
# --------------------------------------------------------------------------

def build_gat_nc(cfg: Cfg, collective: bool = True, iters: int = 1,
                 loop_iters: int = 0, phases: str = "full",
                 debug_dump: bool = False):
    dt = mybir.dt.bfloat16
    f32 = mybir.dt.float32
    add = mybir.AluOpType.add
    mult = mybir.AluOpType.mult
    bypass = mybir.AluOpType.bypass
    Exp = mybir.ActivationFunctionType.Exp
    Lrelu = mybir.ActivationFunctionType.Lrelu

    N, C, HEADS, F2, R = cfg.N, cfg.C, cfg.HEADS, cfg.F2, cfg.R
    JT, CT, HH, CT2, RT = cfg.JT, cfg.CT, cfg.HH, cfg.CT2, cfg.RT
    PW1, PAY, GRP, NG = cfg.PW1, cfg.PAY, cfg.GRP, cfg.NG

    nc = bacc.Bacc(
        "TRN2", target_bir_lowering=False, debug=False, num_devices=cfg.CORES
    )

    # ---- DRAM I/O -------------------------------------------------------
    # wpack = [w1cs (CT*PW1) | wsp (CT*HEADS) | w2a (CT2*PAY) | ident (128)]
    WP0 = CT * PW1
    WP1 = WP0 + CT * HEADS
    WP2 = WP1 + CT2 * PAY
    WPN = WP2 + 128
    xtl_d = nc.dram_tensor("xtloc", [128, CT * R], dt, kind="ExternalInput").ap()
    mb_d = nc.dram_tensor("mb", [128, JT * R], mybir.dt.float8e4,
                         kind="ExternalInput").ap()
    wpack_d = nc.dram_tensor("wpack", [128, WPN], dt, kind="ExternalInput").ap()
    idf_d = nc.dram_tensor("identf", [128, 128], f32, kind="ExternalInput").ap()
    out_d = nc.dram_tensor("out", [R, F2], f32, kind="ExternalOutput").ap()
    if debug_dump:
        dbg_s1b = nc.dram_tensor(
            "dbg_s1b", [128, HEADS * R], dt, kind="ExternalOutput").ap()
        dbg_smin = nc.dram_tensor(
            "dbg_smin", [128, HEADS * JT], f32, kind="ExternalOutput").ap()
        dbg_hloc = nc.dram_tensor(
            "dbg_hloc", [128, CT2 * R], dt, kind="ExternalOutput").ap()
        dbg_wh = nc.dram_tensor(
            "dbg_wh", [128, JT * PW1], dt, kind="ExternalOutput").ap()
        dbg_pagg = nc.dram_tensor(
            "dbg_pagg", [128, HEADS * 2048], f32, kind="ExternalOutput").ap()

    with tile.TileContext(nc) as tc, ExitStack() as ctx:
        const = ctx.enter_context(tc.tile_pool(name="const", bufs=1))
        work = ctx.enter_context(tc.tile_pool(name="work", bufs=3))
        wz = ctx.enter_context(tc.tile_pool(name="wz", bufs=3))
        psA = ctx.enter_context(tc.tile_pool(name="psA", bufs=1, space="PSUM"))
        pss = ctx.enter_context(tc.tile_pool(name="pss", bufs=1, space="PSUM"))
        ps2 = ctx.enter_context(tc.tile_pool(name="ps2", bufs=1, space="PSUM"))
        dram = ctx.enter_context(tc.tile_pool(name="dram", bufs=1, space="DRAM"))

        whsend_t = dram.tile([128, RT * PW1], dt)
        gsend_t = dram.tile([128, RT * PAY], dt)
        if cfg.CORES > 4:
            whfull_t = nc.dram_tensor(
                "whfull_sh", [cfg.CORES * 128, RT * PW1], dt,
                addr_space="Shared").ap()
            gfull_t = nc.dram_tensor(
                "gfull_sh", [cfg.CORES * 128, RT * PAY], dt,
                addr_space="Shared").ap()
        else:
            whfull_t = dram.tile([cfg.CORES * 128, RT * PW1], dt)
            gfull_t = dram.tile([cfg.CORES * 128, RT * PAY], dt)

        import contextlib
        ones_row = const.tile([1, 128], dt)
        nc.vector.memset(ones_row, 1.0)
        loop_cm = (tc.For_i(0, loop_iters, 1) if loop_iters
                   else contextlib.nullcontext())
        with loop_cm:
          _hm = JT * R // 16
          aoff = [0, 512, 1024, 1536]

          def emit_loads():
            cx = {}
            wp_sb = const.tile([128, WPN], dt, tag="wp2", bufs=2, name="wp_sb")
            nc.gpsimd.dma_start(out=wp_sb[:, 0:WP1], in_=wpack_d[:, 0:WP1])
            cx["w1cs"] = wp_sb[:, 0:WP0]
            cx["wsp"] = wp_sb[:, WP0:WP1]
            cx["w2a"] = wp_sb[:, WP1:WP2]
            cx["ident"] = wp_sb[:, WP2:WPN]
            cx["wp"] = wp_sb
            xtl_sb = const.tile([128, CT * R], dt, tag="xtl2", bufs=2,
                                name="xtl_sb")
            nc.gpsimd.dma_start(out=xtl_sb, in_=xtl_d)
            cx["xtl"] = xtl_sb
            mb_sb = const.tile([128, JT * R], mybir.dt.float8e4, tag="mb2",
                               bufs=2, name="mb_sb")
            for _q in range(4):
                nc.gpsimd.dma_start(
                    out=mb_sb[:, _q * _hm: (_q + 1) * _hm],
                    in_=mb_d[:, _q * _hm: (_q + 1) * _hm])
            cx["mb"] = mb_sb
            nc.gpsimd.dma_start(out=wp_sb[:, WP1:WPN], in_=wpack_d[:, WP1:WPN])
            identf_sb = const.tile([128, 128], f32, tag="idf2", bufs=2,
                                   name="identf_sb")
            nc.gpsimd.dma_start(out=identf_sb, in_=idf_d)
            cx["identf"] = identf_sb
            return cx

          def emit_gather(cx):
            w1cs_sb, wsp_sb, xtl_sb, mb_sb = (
                cx["w1cs"], cx["wsp"], cx["xtl"], cx["mb"])
            # local Wh + s- payload; psum matmul outputs stay within one
            # 2KB bank: the 520-wide chain splits at col 512.
            pay_sb = const.tile([128, RT * PW1], dt, name="pay_sb")
            for rt in range(RT):
                pwh = psA.tile([128, PW1], f32, tag="agg", name="pwh")
                for ct in range(CT):
                    lhs = xtl_sb[:, ct * R + rt * 128: ct * R + (rt + 1) * 128]
                    nc.tensor.matmul(
                        out=pwh[:, 0:512], lhsT=lhs,
                        rhs=w1cs_sb[:, ct * PW1: ct * PW1 + 512],
                        start=(ct == 0), stop=(ct == CT - 1),
                    )
                    nc.tensor.matmul(
                        out=pwh[:, 512:PW1], lhsT=lhs,
                        rhs=w1cs_sb[:, ct * PW1 + 512: (ct + 1) * PW1],
                        start=(ct == 0), stop=(ct == CT - 1),
                    )
                pay = pay_sb[:, rt * PW1: (rt + 1) * PW1]
                nc.scalar.copy(out=pay, in_=pwh)
                for h in range(HEADS):
                    nc.vector.memset(pay[:, h * 129 + 128: h * 129 + 129], 1.0)
            nc.sync.dma_start(out=whsend_t, in_=pay_sb)

            # s+ locals: one m=4 chain, flatten rows to partition 0 via the
            # Act DMA queue, broadcast via PE ones-matmul + Act copy.
            psp = pss.tile([HEADS, R], f32, tag="tr", name="psp")
            for ct in range(CT):
                nc.tensor.matmul(
                    out=psp,
                    lhsT=wsp_sb[:, ct * HEADS: (ct + 1) * HEADS],
                    rhs=xtl_sb[:, ct * R: (ct + 1) * R],
                    start=(ct == 0), stop=(ct == CT - 1),
                )
            s1r_sb = const.tile([HEADS, R], dt, tag="s1r2t", bufs=2,
                                name="s1r_sb")
            nc.scalar.copy(out=s1r_sb, in_=psp)
            s1rf_sb = const.tile([1, HEADS * R], dt, tag="s1rf2", bufs=2,
                                 name="s1rf_sb")
            for h in range(HEADS):
                nc.scalar.dma_start(
                    out=s1rf_sb[0:1, h * R: (h + 1) * R],
                    in_=s1r_sb[h: h + 1, :])
            s1b_sb = const.tile([128, HEADS * R], dt, tag="s1b2t", bufs=2,
                                name="s1b_sb")
            for h in range(HEADS):
                pbc = pss.tile([128, R], f32, tag="tr", name="pbc")
                nc.tensor.matmul(out=pbc, lhsT=ones_row,
                                 rhs=s1rf_sb[0:1, h * R: (h + 1) * R],
                                 start=True, stop=True)
                nc.scalar.copy(out=s1b_sb[:, h * R: (h + 1) * R], in_=pbc)
            cx["s1b"] = s1b_sb

            if collective:
                nc.gpsimd.collective_compute(
                    "AllGather", bypass,
                    replica_groups=[list(range(cfg.CORES))],
                    ins=[whsend_t.opt()], outs=[whfull_t.opt()],
                )
            else:
                # timing proxy: 4 send-volume writes carry the cost and the
                # per-chunk dependency; rows beyond 512 are stale (timing
                # values unused).
                for cc in range(4):
                    nc.gpsimd.dma_start(
                        out=whfull_t[cc * 256: cc * 256 + 128, :],
                        in_=whsend_t[:, :])
            # readback in 4 chunked DMAs so L1 starts on early tiles; smin
            # (f32 s- scalars) extracted per chunk on the idle Pool engine.
            wh_sb = const.tile([128, JT * PW1], dt, name="wh_sb")
            smin_sb = const.tile([128, HEADS * JT], f32, tag="smin2", bufs=2,
                                 name="smin_sb")
            whf_v = whfull_t.rearrange("(c p) x -> p c x", p=128)
            for cc in range(4):
                nc.sync.dma_start(
                    out=wh_sb[:, cc * 2 * RT * PW1: (cc + 1) * 2 * RT * PW1]
                    .rearrange("p (c x) -> p c x", c=2),
                    in_=whf_v[:, cc * 2: cc * 2 + 2, :],
                )
                for h in range(HEADS):
                    nc.gpsimd.tensor_copy(
                        out=smin_sb[:, h * JT + cc * 8: h * JT + (cc + 1) * 8]
                        .rearrange("p (t o) -> p t o", o=1),
                        in_=wh_sb.rearrange("p (t q) -> p t q", q=PW1)[
                            :, cc * 8: (cc + 1) * 8,
                            HEADS * 129 + h: HEADS * 129 + h + 1],
                    )
            cx["wh"] = wh_sb
            cx["smin"] = smin_sb
            for _q in range(4, 16):
                nc.scalar.dma_start(
                    out=cx["mb"][:, _q * _hm: (_q + 1) * _hm],
                    in_=mb_d[:, _q * _hm: (_q + 1) * _hm])

          def emit_l1(cx):
            mb_sb, wh_sb, smin_sb, s1b_sb = (
                cx["mb"], cx["wh"], cx["smin"], cx["s1b"])
            ident_sb = cx["ident"]
            hloc_sb = const.tile([128, CT2 * R], dt, tag="hl2", bufs=2,
                                 name="hloc_sb")
            for h in range(HEADS):
                s1b = s1b_sb[:, h * R: (h + 1) * R]
                # one accumulation chain per 2KB PSUM bank (interleaved
                # chains sharing a bank corrupt each other).
                pagg = psA.tile([128, 2048], f32, tag="agg", name="pagg")
                for g in range(NG):
                    ug = wz.tile([128, GRP * R], dt, tag="ug", name="ug")
                    zg = wz.tile([128, GRP * R], dt, tag="zg", name="zg")
                    for k in range(GRP):
                        t = g * GRP + k
                        nc.vector._custom_dve(
                            SCORE_LRELU,
                            out=ug[:, k * R: (k + 1) * R],
                            in0=s1b,
                            in1=mb_sb[:, t * R: (t + 1) * R],
                            s0=smin_sb[:, h * JT + t: h * JT + t + 1],
                            s1=0.0, imm2=ALPHA,
                        )
                    half = GRP * R // 2
                    nc.scalar.activation(out=zg[:, 0:half], in_=ug[:, 0:half],
                                         func=Exp)
                    nc.scalar.activation(out=zg[:, half:], in_=ug[:, half:],
                                         func=Exp)
                    for k in range(GRP):
                        t = g * GRP + k
                        for i4 in range(4):
                            nc.tensor.matmul(
                                out=pagg[:, aoff[i4]: aoff[i4] + 129],
                                lhsT=zg[:, k * R + i4 * 128:
                                        k * R + (i4 + 1) * 128],
                                rhs=wh_sb[:, t * PW1 + h * 129:
                                          t * PW1 + (h + 1) * 129],
                                start=(t == 0), stop=(t == JT - 1),
                            )
                # normalize + elu + transpose -> hloc tile for this head
                for i4 in range(4):
                    rcp = work.tile([128, 1], f32, tag="rcp", name="rcp")
                    nc.vector.reciprocal(
                        out=rcp, in_=pagg[:, aoff[i4] + 128: aoff[i4] + 129])
                    hu = work.tile([128, 128], f32, tag="hu", name="hu")
                    nc.scalar.activation(
                        out=hu, in_=pagg[:, aoff[i4]: aoff[i4] + 128],
                        func=mybir.ActivationFunctionType.Copy, scale=rcp,
                    )
                    eh = work.tile([128, 128], dt, tag="eh", name="eh")
                    nc.scalar.activation(out=eh, in_=hu, func=Exp)
                    helu = work.tile([128, 128], dt, tag="helu", name="helu")
                    nc.vector._custom_dve(
                        ELU_SEL, out=helu, in0=hu, in1=eh,
                        s0=1.0, s1=0.0, imm2=0.0,
                    )
                    pT = pss.tile([128, 128], dt, tag="tr", name="pT")
                    nc.tensor.transpose(out=pT, in_=helu, identity=ident_sb)
                    nc.scalar.copy(
                        out=hloc_sb[:, h * R + i4 * 128:
                                    h * R + (i4 + 1) * 128],
                        in_=pT)
            cx["hloc"] = hloc_sb

          def emit_l2fin(cx):
            mb_sb, hloc_sb, ident_sb = cx["mb"], cx["hloc"], cx["ident"]
            w2a_sb, identf_sb, s1b_sb = cx["w2a"], cx["identf"], cx["s1b"]
            # layer-2 local projections + gather payload
            # w2a cols per ct2: [W2(64)|w2p|w2m|pad]; pW = [Wh2 | s+ | s-]
            sp2_sb = work.tile([128, RT], dt, tag="sp2", name="sp2_sb")
            gs_sb = const.tile([128, RT * PAY], dt, name="gs_sb")
            for rt in range(RT):
                pW = pss.tile([128, PAY], f32, tag="pw", bufs=2, name="pW")
                for ct in range(CT2):
                    nc.tensor.matmul(
                        out=pW,
                        lhsT=hloc_sb[:, ct * R + rt * 128:
                                     ct * R + (rt + 1) * 128],
                        rhs=w2a_sb[:, ct * PAY: (ct + 1) * PAY],
                        start=(ct == 0), stop=(ct == CT2 - 1),
                    )
                gs = gs_sb[:, rt * PAY: (rt + 1) * PAY]
                nc.scalar.copy(out=gs[:, 0:F2], in_=pW[:, 0:F2])
                nc.vector.memset(gs[:, F2: F2 + 1], 1.0)
                nc.vector.tensor_copy(
                    out=gs[:, F2 + 1: F2 + 2], in_=pW[:, F2 + 1: F2 + 2])
                nc.vector.tensor_copy(
                    out=sp2_sb[:, rt: rt + 1], in_=pW[:, F2: F2 + 1])
            nc.sync.dma_start(out=gsend_t, in_=gs_sb)

            # local s+ -> s1b2 broadcast (no gather dependency)
            s1r2f_sb = const.tile([1, R], dt, name="s1r2f_sb")
            for rt in range(RT):
                pt1 = pss.tile([1, 128], dt, tag="tr", name="pt1")
                nc.tensor.transpose(
                    out=pt1, in_=sp2_sb[:, rt: rt + 1], identity=ident_sb)
                nc.vector.tensor_copy(
                    out=s1r2f_sb[0:1, rt * 128: (rt + 1) * 128], in_=pt1)
            pbc2 = pss.tile([128, R], f32, tag="tr", name="pbc2")
            nc.tensor.matmul(out=pbc2, lhsT=ones_row, rhs=s1r2f_sb,
                             start=True, stop=True)
            s1b2_sb = const.tile([128, R], dt, name="s1b2_sb")
            nc.scalar.copy(out=s1b2_sb, in_=pbc2)

            if collective:
                nc.gpsimd.collective_compute(
                    "AllGather", bypass,
                    replica_groups=[list(range(cfg.CORES))],
                    ins=[gsend_t.opt()], outs=[gfull_t.opt()],
                )
            else:
                nc.gpsimd.dma_start(
                    out=gfull_t[0:128, :], in_=gsend_t[:, :])

            gf_sb = const.tile([128, JT * PAY], dt, name="gf_sb")
            gff_v = gfull_t.rearrange("(c p) x -> p c x", p=128)
            _gfq = [nc.sync, nc.scalar, nc.sync, nc.scalar]
            for cc in range(4):
                _gfq[cc].dma_start(
                    out=gf_sb[:, cc * 2 * RT * PAY: (cc + 1) * 2 * RT * PAY]
                    .rearrange("p (c x) -> p c x", c=2),
                    in_=gff_v[:, cc * 2: cc * 2 + 2, :],
                )
            s2pf = const.tile([128, JT], f32, name="s2pf")
            nc.vector.tensor_copy(
                out=s2pf[:, :].rearrange("p (t o) -> p t o", o=1),
                in_=gf_sb.rearrange("p (t q) -> p t q", q=PAY)[
                    :, :, F2 + 1: F2 + 2],
            )

            # layer-2 attention + aggregation
            psum2 = ps2.tile([F2 + 1, R], f32, name="psum2")
            for g in range(NG):
                ug = wz.tile([128, GRP * R], dt, tag="ug", name="ug2")
                zg = wz.tile([128, GRP * R], dt, tag="zg", name="zg2")
                for k in range(GRP):
                    t = g * GRP + k
                    nc.vector._custom_dve(
                        SCORE_LRELU,
                        out=ug[:, k * R: (k + 1) * R],
                        in0=s1b2_sb,
                        in1=mb_sb[:, t * R: (t + 1) * R],
                        s0=s2pf[:, t: t + 1], s1=0.0, imm2=ALPHA,
                    )
                half = GRP * R // 2
                nc.scalar.activation(out=zg[:, 0:half], in_=ug[:, 0:half],
                                     func=Exp)
                nc.scalar.activation(out=zg[:, half:], in_=ug[:, half:],
                                     func=Exp)
                for k in range(GRP):
                    t = g * GRP + k
                    nc.tensor.matmul(
                        out=psum2,
                        lhsT=gf_sb[:, t * PAY: t * PAY + F2 + 1],
                        rhs=zg[:, k * R: (k + 1) * R],
                        start=(t == 0), stop=(t == JT - 1),
                    )

            # finalize: transpose, normalize, store
            o2 = const.tile([F2 + 1, R], f32, name="o2")
            nc.scalar.copy(out=o2, in_=psum2)
            of_sb = const.tile([128, RT * F2], f32, name="of_sb")
            for rt in range(RT):
                pT2 = pss.tile([128, F2 + 1], f32, tag="pw", bufs=2,
                               name="pT2")
                nc.tensor.transpose(
                    out=pT2,
                    in_=o2[:, rt * 128: (rt + 1) * 128],
                    identity=identf_sb[0: F2 + 1, 0: F2 + 1],
                )
                rc = work.tile([128, 1], f32, tag="rc", name="rc")
                nc.vector.reciprocal(out=rc, in_=pT2[:, F2: F2 + 1])
                nc.vector.tensor_scalar(
                    out=of_sb[:, rt * F2: (rt + 1) * F2],
                    in0=pT2[:, 0:F2], scalar1=rc, scalar2=0.0,
                    op0=mult, op1=bypass,
                )
            nc.sync.dma_start(
                out=out_d.rearrange("(rt p) f -> p rt f", p=128),
                in_=of_sb.rearrange("p (rt f) -> p rt f", rt=RT),
            )

          def dummy_store(cx):
            for rt in range(RT):
                nc.sync.dma_start(out=out_d[rt * 128:(rt + 1) * 128, :],
                                  in_=cx["identf"][:, 0:F2])

          # prologues for all unrolled iterations first, so the second
          # iteration's loads / payload / gather hide under the first
          # iteration's attention phases.
          cxs = []
          for _it in range(iters):
              cx = emit_loads()
              if phases == "dma":
                  dummy_store(cx)
                  continue
              emit_gather(cx)
              cxs.append(cx)
          if phases == "wh":
              for cx in cxs:
                  dummy_store(cx)
              cxs = []
          for cx in cxs:
              emit_l1(cx)
          for cx in cxs:
              if phases == "l1":
                  dummy_store(cx)
                  continue
              emit_l2fin(cx)

    nc.compile()
    return nc


# --------------------------------------------------------------------------
# Host-side prep / sharding
# --------------------------------------------------------------------------

def host_prep(cfg: Cfg, g, inputs, W1, a1, W2, a2):
    N, C, H, HEADS, F2, R = cfg.N, cfg.C, cfg.H, cfg.HEADS, cfg.F2, cfg.R
    CT, CT2, PW1, PAY = cfg.CT, cfg.CT2, cfg.PW1, cfg.PAY
    X = np.asarray(inputs, np.float32)
    W1 = np.asarray(W1, np.float32)
    a1 = np.asarray(a1, np.float32)
    W2 = np.asarray(W2, np.float32)
    a2 = np.asarray(a2, np.float32)

    def tile128(A):
        # [k*128, cols] row-major -> partition-major [128, k*cols]
        k = A.shape[0] // 128
        return np.ascontiguousarray(
            A.reshape(k, 128, A.shape[1]).transpose(1, 0, 2).reshape(128, -1)
        )

    XT = np.ascontiguousarray(X.T)                                    # [C, N]

    # w1cs: per ct block [128, PW1]: [W1_h0 |0| W1_h1 |0| W1_h2 |0| W1_h3 |0|
    #                                 ws1m_0..3]
    ws1p = np.stack([W1[h] @ a1[h, :H, 0] for h in range(HEADS)], axis=1)  # [C,4]
    ws1m = np.stack([W1[h] @ a1[h, H:, 0] for h in range(HEADS)], axis=1)  # [C,4]
    w1cs = np.zeros((C, PW1), np.float32)
    for h in range(HEADS):
        w1cs[:, h * 129: h * 129 + 128] = W1[h]
    w1cs[:, HEADS * 129:] = ws1m
    w1cs_t = tile128(w1cs.astype(BF16))
    wsp_t = tile128(ws1p.astype(BF16))

    # w2a: per ct2 block [128, PAY]: [W2(64) | w2p | w2m | pad]
    w2p = W2 @ a2[:F2, 0]                                             # [HH]
    w2m = W2 @ a2[F2:, 0]                                             # [HH]
    w2a = np.zeros((HEADS * H, PAY), np.float32)
    w2a[:, :F2] = W2
    w2a[:, F2] = w2p
    w2a[:, F2 + 1] = w2m
    w2a_t = tile128(w2a.astype(BF16))

    ident = np.eye(128, dtype=BF16)
    identf = np.eye(128, dtype=np.float32)
    wpack = np.concatenate([w1cs_t, wsp_t, w2a_t, ident], axis=1)

    adj = np.asarray(g) > 0
    in_maps = []
    for c in range(cfg.CORES):
        rows = slice(c * R, (c + 1) * R)
        mb = np.where(adj[rows].T, 0.0, MASKBIAS).astype(
            ml_dtypes.float8_e4m3)                                    # [N, R]
        in_maps.append({
            "xtloc": tile128(np.ascontiguousarray(XT[:, rows]).astype(BF16)),
            "mb": tile128(np.ascontiguousarray(mb)),
            "wpack": wpack, "identf": identf,
        })
    return in_maps


_NC_CACHE = {}


def get_compiled(cfg: Cfg):
    nc = _NC_CACHE.get(cfg)
    if nc is None:
        nc = build_gat_nc(cfg)
        _NC_CACHE[cfg] = nc
    return nc


def kernel(g, inputs, W1, a1, W2, a2):
    cfg = FULL
    nc = get_compiled(cfg)
    in_maps = host_prep(cfg, g, inputs, W1, a1, W2, a2)
    res = run_bass_kernel_spmd(nc, in_maps, core_ids=list(range(cfg.CORES)))
    out = np.concatenate(
        [np.asarray(res.results[c]["out"], np.float32) for c in range(cfg.CORES)],
        axis=0,
    )
    return out
